# revision 36
# baseline (speedup 1.0000x reference)
"""Multi-head attention (B=1, S=2048, H=1024, NH=16) on 8 trn2 NeuronCores.

Sharding: head-parallel. Core c owns heads {2c, 2c+1} (= 128 of the 1024
hidden dims). Each core computes its Q/K/V projection slices, the full
attention for its 2 heads, and a full-width partial of the output
projection (contraction over its 128 context dims). Host sums the 8
partials and adds the (host-folded) biases.

v4 schedule - the Act engine is the pacemaker and nothing gates it:
  - The reference's masked softmax is exp(s*M) with masked scores set to
    0 (so masked weights are exp(0)=1).  We decompose
        exp(s*M) = (exp(s) - 1)*M + 1:
    Act computes plain exp(s/8) STRAIGHT FROM PSUM (no mask needed ->
    64 back-to-back 1024-wide exps, the critical 66us), DVE/Pool apply
    (E*M - M) in all-SBUF bf16 (DVE tensor_tensor runs 2x there), and
    the "+1" term enters the PV accumulation as a host-precomputed
    rank-1 matmul C = [colsum(V_proj) | 2048] per head.
  - q/k arrive as 512-token panels, projections panel-wise into one
    rotating PSUM bank (bias folded into evictions: Act for the first
    three panels, DVE after), k panels before masks (Act's S inputs).
  - GPSIMD cannot touch PSUM, so Pool only gets SBUF work: the tail of
    the mask-subtract, vaug ones columns.
  - per (h,j): S (4x512 PE into 2 psum tiles) -> 2 exps (Act) ->
    et=E*M (DVE 2x) -> et-=M (DVE [0:1152] + Pool [1152:2048]) ->
    PV (65-wide, ones-column denominator), software-pipelined by one j.
  - h0 epilogue (normalize) rides h1's slots; transposes/oT/y pipeline
    in the tail across aux/o-pool psum with Act+DVE evictions and eager
    per-chunk output DMA.

Precision: matmuls bf16 with fp32 PSUM accumulation; 0/1 mask bf16
(exact). Softmax runs without max-subtraction: exponent is (q.k/8) ~
N(0, 0.33^2) so exp never overflows; et = (E-1)*M is in [-1, 7].
"""

import math

import numpy as np
import ml_dtypes

BF16 = ml_dtypes.bfloat16
S, H, NH, DK = 2048, 1024, 16, 64
NCORES = 8
HPC = NH // NCORES          # heads per core = 2
DPC = HPC * DK              # head dims per core = 128
KC = H // 128               # contraction chunks = 8
TP = S // 512               # 512-wide token panels = 4
JC = S // 128               # 128-wide key chunks = 16
VA = DK + 1                 # v columns + ones column = 65
NWARM = 12                  # PE warm-up matmuls
DSUB = 1216                 # DVE's share of the mask-subtract columns

_CACHE = {}


def _oslc(ic):
    """o_ps column offset for ic-th 65-wide slice: 7 slices per 512-fp32
    PSUM bank so no matmul crosses a bank boundary."""
    b, r = divmod(ic, 7)
    return b * 512 + r * VA


def _build_program():
    """Build + compile the (identical) per-core Bass program."""
    from contextlib import ExitStack

    import concourse.bacc as bacc
    import concourse.bass as bass_mod
    import concourse.tile as tile
    from concourse import mybir

    dt = mybir.dt
    AF = mybir.ActivationFunctionType
    ALU = mybir.AluOpType

    nc = bacc.Bacc("TRN2", target_bir_lowering=False, debug=False)

    qT_d = nc.dram_tensor("qT", [H, S], dt.bfloat16, kind="ExternalInput").ap()
    kT_d = nc.dram_tensor("kT", [H, S], dt.bfloat16, kind="ExternalInput").ap()
    vT_d = nc.dram_tensor("vT", [H, S], dt.bfloat16, kind="ExternalInput").ap()
    maskT_d = nc.dram_tensor("maskT", [S, S], dt.bfloat16, kind="ExternalInput").ap()
    # wpack1 = [ident | wq | wk], wpack2 = [wv | wo]
    wp1_d = nc.dram_tensor("wp1", [128, 128 + 2 * KC * DPC], dt.bfloat16,
                           kind="ExternalInput").ap()
    wp2_d = nc.dram_tensor("wp2", [128, KC * DPC + H], dt.bfloat16,
                           kind="ExternalInput").ap()
    bq_d = nc.dram_tensor("bq", [DPC, 1], dt.float32, kind="ExternalInput").ap()
    bk_d = nc.dram_tensor("bk", [DPC, 1], dt.float32, kind="ExternalInput").ap()
    caug_d = nc.dram_tensor("caug", [33, VA], dt.bfloat16, kind="ExternalInput").ap()
    yT_d = nc.dram_tensor("yT", [H, S], dt.bfloat16, kind="ExternalOutput").ap()

    with tile.TileContext(nc) as tc, ExitStack() as ctx:
        cp = ctx.enter_context(tc.tile_pool(name="const", bufs=1))
        xin_p = ctx.enter_context(tc.tile_pool(name="xin", bufs=4))
        vin_p = ctx.enter_context(tc.tile_pool(name="vin", bufs=3))
        e_p = ctx.enter_context(tc.tile_pool(name="ex", bufs=12))
        ot_p = ctx.enter_context(tc.tile_pool(name="otok", bufs=2))
        rc_p = ctx.enter_context(tc.tile_pool(name="recip", bufs=3))
        y_p = ctx.enter_context(tc.tile_pool(name="ysb", bufs=6))
        # PSUM: aux 1 bank (warmup/proj/pv/tp/y-singles), s 2x2 banks
        # (S tiles, then y pairs), o 3 banks (PV acc, then tp scratch)
        aux_p = ctx.enter_context(tc.tile_pool(name="ps_aux", bufs=1, space="PSUM"))
        s_p = ctx.enter_context(tc.tile_pool(name="ps_s", bufs=2, space="PSUM"))
        o_p = ctx.enter_context(tc.tile_pool(name="ps_o", bufs=1, space="PSUM"))

        wp1 = cp.tile([128, 128 + 2 * KC * DPC], dt.bfloat16, tag="wp1")
        nc.sync.dma_start(out=wp1, in_=wp1_d)
        ident = wp1[:, 0:128]
        w_q = wp1[:, 128 : 128 + KC * DPC]
        w_k = wp1[:, 128 + KC * DPC : 128 + 2 * KC * DPC]
        bq_sb = cp.tile([DPC, 1], dt.float32, tag="bq")
        nc.sync.dma_start(out=bq_sb, in_=bq_d)
        bk_sb = cp.tile([DPC, 1], dt.float32, tag="bk")
        nc.sync.dma_start(out=bk_sb, in_=bk_d)
        caug_sb = cp.tile([33, VA], dt.bfloat16, tag="caug")
        nc.sync.dma_start(out=caug_sb, in_=caug_d)

        qT_sb = cp.tile([128, S], dt.bfloat16, tag="qTs")
        kT_sb = cp.tile([128, S], dt.bfloat16, tag="kTs")
        vaug = cp.tile([128, JC * (HPC * VA)], dt.bfloat16, tag="vaug")
        m_sb = [cp.tile([128, S], dt.bfloat16, tag=f"mj{j}", name=f"mj{j}")
                for j in range(JC)]
        oT_sb = [cp.tile([128, 512], dt.bfloat16, tag=f"oTp{p}", name=f"oTp{p}")
                 for p in range(TP)]
        ones1 = cp.tile([33, 128], dt.bfloat16, tag="ones1")
        nc.vector.memset(ones1, 1.0)

        xin = {}
        vin8 = [None] * KC

        def dma_panel(pre, x_d, p):
            xt = xin_p.tile([128, KC * 512], dt.bfloat16, tag="xin",
                            name=f"x{pre}{p}")
            nc.sync.dma_start(
                out=xt.rearrange("p (c i) -> p c i", c=KC),
                in_=x_d[:, p * 512 : (p + 1) * 512].rearrange(
                    "(c p) i -> p c i", p=128
                ),
            )
            xin[pre, p] = xt

        def dma_mask(j):
            nc.sync.dma_start(out=m_sb[j], in_=maskT_d[j * 128 : (j + 1) * 128, :])

        def dma_v(g):
            vt = vin_p.tile([128, KC * 256], dt.bfloat16, tag="vin", name=f"v{g}")
            nc.sync.dma_start(
                out=vt.rearrange("p (c i) -> p c i", c=KC),
                in_=vT_d[:, g * 256 : (g + 1) * 256].rearrange(
                    "(c p) i -> p c i", p=128
                ),
            )
            vin8[g] = vt

        # DMA order: all q/k panels first (they gate Act's exp stream),
        # then wv/wo, then masks and v tiles streaming (consumed by the
        # trailing DVE/Pool mask-apply + PV, which have slack).
        dma_panel("q", qT_d, 0)
        dma_panel("q", qT_d, 1)
        dma_panel("k", kT_d, 0)
        dma_panel("q", qT_d, 2)
        dma_panel("q", qT_d, 3)
        dma_panel("k", kT_d, 1)
        dma_panel("k", kT_d, 2)
        dma_panel("k", kT_d, 3)
        wp2 = cp.tile([128, KC * DPC + H], dt.bfloat16, tag="wp2")
        nc.sync.dma_start(out=wp2, in_=wp2_d)
        w_v = wp2[:, 0 : KC * DPC]
        wo_sb = wp2[:, KC * DPC : KC * DPC + H]
        dma_v(0)
        dma_mask(0)
        dma_mask(1)
        dma_v(1)
        dma_mask(2)
        dma_mask(3)
        dma_v(2)
        dma_mask(4)
        dma_mask(5)
        dma_v(3)
        dma_mask(6)
        dma_mask(7)
        dma_v(4)
        dma_mask(8)
        dma_mask(9)
        dma_v(5)
        dma_mask(10)
        dma_mask(11)
        dma_v(6)
        dma_mask(12)
        dma_mask(13)
        dma_v(7)
        dma_mask(14)
        dma_mask(15)

        # ones columns of vaug (Pool, SBUF-only)
        ones_cols = bass_mod.AP(
            tensor=vaug.tensor,
            offset=vaug.offset + DK,
            ap=[vaug.ap[0], [VA, JC * HPC], [1, 1]],
        )
        nc.gpsimd.memset(ones_cols, 1.0)

        # PE warm-up out of the cold p-state while DMAs stream
        for wi in range(NWARM):
            wps = aux_p.tile([128, 128], dt.float32, tag="aux", name=f"warm{wi}")
            nc.tensor.matmul(wps, lhsT=ident, rhs=ident, start=True, stop=True)

        # ---- panel-wise projections; first three evict on Act (they
        # precede the exp stream), later ones on DVE ----
        def proj_panel(pre, w_ap, b_sb, dest, p, act_evict):
            ps = aux_p.tile([128, 512], dt.float32, tag="aux", name=f"pp{pre}{p}")
            for kk in range(KC):
                nc.tensor.matmul(
                    ps,
                    lhsT=w_ap[:, kk * DPC : (kk + 1) * DPC],
                    rhs=xin[pre, p][:, kk * 512 : (kk + 1) * 512],
                    start=(kk == 0),
                    stop=(kk == KC - 1),
                )
            dst = dest[:, p * 512 : (p + 1) * 512]
            if act_evict:
                nc.scalar.activation(dst, ps, AF.Identity, bias=b_sb, scale=1.0)
            else:
                nc.vector.tensor_scalar(dst, ps, b_sb, None, ALU.add)

        def v_proj_chunk(t):
            ps = aux_p.tile([128, DPC], dt.float32, tag="aux", name=f"pv{t}")
            g, half = divmod(t, 2)
            for kk in range(KC):
                nc.tensor.matmul(
                    ps,
                    lhsT=vin8[g][:, kk * 256 + half * 128 : kk * 256 + half * 128 + 128],
                    rhs=w_v[:, kk * DPC : (kk + 1) * DPC],
                    start=(kk == 0),
                    stop=(kk == KC - 1),
                )
            # both heads' 64-col slices in one strided DVE copy
            base = t * (HPC * VA)
            dst = bass_mod.AP(
                tensor=vaug.tensor,
                offset=vaug.offset + base,
                ap=[vaug.ap[0], [VA, HPC], [1, DK]],
            )
            nc.vector.tensor_copy(dst, ps.rearrange("p (a d) -> p a d", d=DK))

        proj_panel("q", w_q, bq_sb, qT_sb, 0, True)
        proj_panel("q", w_q, bq_sb, qT_sb, 1, True)
        proj_panel("k", w_k, bk_sb, kT_sb, 0, True)

        # ---- attention ----
        import itertools

        def emit_exps(h, j, et):
            """S^T chunk j (PE) + the two 1024-wide exps (Act, from PSUM)."""
            hs = h * DK
            for half in range(2):
                ps = s_p.tile([128, 1024], dt.float32, tag="sps",
                              name=f"s{h}_{j}_{half}")
                for q in range(2):
                    pi = half * 2 + q
                    nc.tensor.matmul(
                        ps[:, q * 512 : (q + 1) * 512],
                        lhsT=kT_sb[hs : hs + DK, j * 128 : (j + 1) * 128],
                        rhs=qT_sb[hs : hs + DK, pi * 512 : (pi + 1) * 512],
                        start=True,
                        stop=True,
                    )
                nc.scalar.activation(
                    et[:, half * 1024 : (half + 1) * 1024], ps, AF.Exp,
                    scale=1.0 / math.sqrt(DK),
                )

        def emit_mask(h, j, et):
            """et = et*M - M, all-SBUF bf16 (DVE tensor_tensor runs 2x).
            Pool takes more of the subtract during h1 so DVE never trails
            the exp stream at the end."""
            mj = m_sb[j]
            ds = 1200
            nc.vector.tensor_tensor(et, et, mj, ALU.mult)
            nc.vector.tensor_tensor(
                et[:, 0:ds], et[:, 0:ds], mj[:, 0:ds], ALU.subtract
            )
            nc.gpsimd.tensor_tensor(
                et[:, ds:S], et[:, ds:S], mj[:, ds:S], ALU.subtract
            )

        def pv_c(h, o_ps):
            """+1 correction: rank-1 C rows into every o slice."""
            for ic in range(JC):
                nc.tensor.matmul(
                    o_ps[:, _oslc(ic) : _oslc(ic) + VA],
                    lhsT=ones1[32 * h : 32 * h + 1, :],
                    rhs=caug_sb[32 * h : 32 * h + 1, :],
                    start=(ic % 7 == 0),
                    stop=False,
                )

        def pv_mms(h, j, et, o_ps):
            for ic in range(JC):
                nc.tensor.matmul(
                    o_ps[:, _oslc(ic) : _oslc(ic) + VA],
                    lhsT=et[:, ic * 128 : (ic + 1) * 128],
                    rhs=vaug[:, j * (HPC * VA) + h * VA : j * (HPC * VA) + (h + 1) * VA],
                    start=False,
                    stop=(j == JC - 1 and (ic % 7 == 6 or ic == JC - 1)),
                )

        epi_q = []

        def norm_bank(h, o_ps, b, ot_big):
            n_ic = (7, 7, 2)[b]
            rc = rc_p.tile([128, 8], dt.float32, tag="rc", name=f"rc{h}_{b}")
            den = bass_mod.AP(
                tensor=o_ps.tensor,
                offset=o_ps.offset + b * 512 + DK,
                ap=[o_ps.ap[0], [VA, n_ic]],
            )
            nc.vector.reciprocal(rc[:, :n_ic], den)
            src_ap = bass_mod.AP(
                tensor=o_ps.tensor,
                offset=o_ps.offset + b * 512,
                ap=[o_ps.ap[0], [VA, n_ic], [1, DK]],
            )
            rcb = bass_mod.AP(
                tensor=rc.tensor,
                offset=rc.offset,
                ap=[rc.ap[0], [1, n_ic], [0, DK]],
            )
            dst = ot_big[:, b * 7 * DK : (b * 7 + n_ic) * DK].rearrange(
                "p (a d) -> p a d", d=DK
            )
            nc.vector.tensor_mul(dst, src_ap, rcb)

        def tp_pair(h, ic, ot_big, tp_dst, dve_both=False):
            """Transpose O chunk pair (ic, ic+1) [128,128]; copy the two
            64-row halves into oT_sb (DVE + Act, or both DVE when riding
            h1's slots where Act is the pacemaker)."""
            hs = h * DK
            ot = ot_big[:, ic * DK : (ic + 2) * DK]
            nc.tensor.transpose(tp_dst, ot, ident)
            p = ic // 4
            dst0 = oT_sb[p][hs : hs + DK, (ic % 4) * 128 : (ic % 4 + 1) * 128]
            dst1 = oT_sb[p][hs : hs + DK, (ic % 4 + 1) * 128 : (ic % 4 + 2) * 128]
            nc.vector.tensor_copy(dst0, tp_dst[0:DK, :])
            if dve_both:
                nc.vector.tensor_copy(dst1, tp_dst[DK : 2 * DK, :])
            else:
                nc.scalar.activation(dst1, tp_dst[DK : 2 * DK, :], AF.Copy)

        def tp_pair_aux0(ic):
            tp = aux_p.tile([128, 128], dt.bfloat16, tag="aux", name=f"tpe0_{ic}")
            tp_pair(0, ic, ot_bigs[0], tp, dve_both=True)

        def emit_exp_half(h, j, et, half):
            hs = h * DK
            ps = s_p.tile([128, 1024], dt.float32, tag="sps",
                          name=f"s{h}_{j}_{half}")
            for q in range(2):
                pi = half * 2 + q
                nc.tensor.matmul(
                    ps[:, q * 512 : (q + 1) * 512],
                    lhsT=kT_sb[hs : hs + DK, j * 128 : (j + 1) * 128],
                    rhs=qT_sb[hs : hs + DK, pi * 512 : (pi + 1) * 512],
                    start=True,
                    stop=True,
                )
            nc.scalar.activation(
                et[:, half * 1024 : (half + 1) * 1024], ps, AF.Exp,
                scale=1.0 / math.sqrt(DK),
            )

        o_ps_h = [None, None]
        ot_bigs = [None, None]
        for h in range(HPC):
            o_ps = o_p.tile([128, 1536], dt.float32, tag="ops", name=f"ops{h}")
            o_ps_h[h] = o_ps
            pv_c(h, o_ps)
            pend = []  # (j, et) with exps emitted, mask+PV pending
            if h == 0:
                # warm start: half0 of j0/j1 (need only q panels 0-1 + k0),
                # then the deferred projections ride between the exps
                ets = {}
                for j in (0, 1):
                    ets[j] = e_p.tile([128, S], dt.bfloat16, tag="et",
                                      name=f"et0_{j}")
                    emit_exp_half(0, j, ets[j], 0)
                proj_panel("q", w_q, bq_sb, qT_sb, 2, True)
                proj_panel("q", w_q, bq_sb, qT_sb, 3, True)
                for j in (0, 1):
                    emit_exp_half(0, j, ets[j], 1)
                    pend.append((j, ets[j]))
                proj_panel("k", w_k, bk_sb, kT_sb, 1, False)
                proj_panel("k", w_k, bk_sb, kT_sb, 2, False)
                proj_panel("k", w_k, bk_sb, kT_sb, 3, False)
                jstart = 2
            else:
                jstart = 0
            for j in range(jstart, JC):
                et = e_p.tile([128, S], dt.bfloat16, tag="et", name=f"et{h}_{j}")
                emit_exps(h, j, et)
                if pend:
                    pj, pet = pend.pop(0)
                    emit_mask(h, pj, pet)
                    if h == 0:
                        v_proj_chunk(pj)
                    else:
                        if epi_q:
                            epi_q.pop(0)()
                    pv_mms(h, pj, pet, o_ps)
                pend.append((j, et))
            for pj, pet in pend:
                emit_mask(h, pj, pet)
                if h == 0:
                    v_proj_chunk(pj)
                pv_mms(h, pj, pet, o_ps)
            ot_bigs[h] = ot_p.tile([128, JC * DK], dt.bfloat16, tag="ot",
                                   name=f"otb{h}")
            if h == 0:
                for b in range(3):
                    epi_q.append(
                        lambda b=b: norm_bank(0, o_ps_h[0], b, ot_bigs[0])
                    )
                for ic in range(0, JC, 2):
                    epi_q.append(lambda ic=ic: tp_pair_aux0(ic))
            else:
                while epi_q:
                    epi_q.pop(0)()
                for b in range(3):
                    norm_bank(1, o_ps_h[1], b, ot_bigs[1])

        # ---- tail: per panel-half, transposes for both heads (aux /
        # o-scratch alternating psum, copies split DVE/Act) immediately
        # followed by that half's y chunks over 4 psum streams ----
        flip = itertools.cycle((True, True, False))
        tpi = 0

        def do_tp(h, ic):
            if (ic // 2 + h) % 2 == 0:
                tp = aux_p.tile([128, 128], dt.bfloat16, tag="aux",
                                name=f"tp{h}_{ic}")
            else:
                tp = o_p.tile([128, 128], dt.bfloat16, tag="ops",
                              name=f"tp{h}_{ic}")
            tp_pair(h, ic, ot_bigs[h], tp)

        for ph in range(2):
            for ic in range(8 * ph, 8 * ph + 8, 2):
                do_tp(1, ic)
            for nn in range(KC):
                ysb = y_p.tile([128, 1024], dt.bfloat16, tag="ysb",
                               name=f"ysb{ph}_{nn}")
                if nn % 4 == 3:
                    # singles on aux + o-pool rotations
                    for pp in range(2):
                        if pp == 0:
                            y_ps = aux_p.tile([128, 512], dt.float32,
                                              tag="aux", name=f"ya{ph}_{nn}")
                        else:
                            y_ps = o_p.tile([128, 512], dt.float32,
                                            tag="ops", name=f"yo{ph}_{nn}")
                        nc.tensor.matmul(
                            y_ps,
                            lhsT=wo_sb[:, nn * 128 : (nn + 1) * 128],
                            rhs=oT_sb[ph * 2 + pp],
                            start=True,
                            stop=True,
                        )
                        if next(flip):
                            nc.scalar.activation(
                                ysb[:, pp * 512 : (pp + 1) * 512], y_ps, AF.Copy
                            )
                        else:
                            nc.vector.tensor_copy(
                                ysb[:, pp * 512 : (pp + 1) * 512], y_ps
                            )
                else:
                    y_ps = s_p.tile([128, 1024], dt.float32, tag="sps",
                                    name=f"y{ph}_{nn}")
                    for pp in range(2):
                        nc.tensor.matmul(
                            y_ps[:, pp * 512 : (pp + 1) * 512],
                            lhsT=wo_sb[:, nn * 128 : (nn + 1) * 128],
                            rhs=oT_sb[ph * 2 + pp],
                            start=True,
                            stop=True,
                        )
                    if next(flip):
                        nc.scalar.activation(ysb, y_ps, AF.Copy)
                    else:
                        nc.vector.tensor_copy(ysb, y_ps)
                nc.sync.dma_start(
                    out=yT_d[nn * 128 : (nn + 1) * 128,
                             ph * 1024 : ph * 1024 + 1024],
                    in_=ysb,
                )

    nc.compile()
    return nc


def get_program():
    if "nc" not in _CACHE:
        _CACHE["nc"] = _build_program()
    return _CACHE["nc"]


def _wshuf(wT):
    """[1024 k, 128 n] -> [128 p, KC*128] with chunk kk at cols kk*128."""
    return np.ascontiguousarray(
        wT.reshape(KC, 128, DPC).transpose(1, 0, 2).reshape(128, KC * DPC)
    ).astype(BF16)


def make_in_maps(query, key, value, attention_mask, Wq, bq, Wk, bk, Wv, Wo):
    """Host-side sharding: per-core input dicts."""
    qT = np.ascontiguousarray(np.asarray(query, np.float32)[0].T).astype(BF16)
    kT = np.ascontiguousarray(np.asarray(key, np.float32)[0].T).astype(BF16)
    vT = np.ascontiguousarray(np.asarray(value, np.float32)[0].T).astype(BF16)
    maskT = np.ascontiguousarray(
        np.asarray(attention_mask, np.float32)[0, 0].T
    ).astype(BF16)
    v32 = np.asarray(value, np.float32)[0]
    vcol = v32.sum(axis=0)  # [H]

    in_maps = []
    for c in range(NCORES):
        ns = slice(c * DPC, (c + 1) * DPC)
        wq = _wshuf(np.asarray(Wq, np.float32)[ns].T)
        wk = _wshuf(np.asarray(Wk, np.float32)[ns].T)
        wv = _wshuf(np.asarray(Wv, np.float32)[ns].T)
        wo = np.ascontiguousarray(np.asarray(Wo, np.float32)[:, ns].T).astype(BF16)
        wp1 = np.concatenate([np.eye(128, dtype=BF16), wq, wk], axis=1)
        wp2 = np.concatenate([wv, wo], axis=1)
        cvec = vcol @ np.asarray(Wv, np.float32)[ns].T  # [DPC]
        caug = np.zeros((33, VA), np.float32)
        for hh in range(HPC):
            caug[32 * hh, 0:DK] = cvec[hh * DK : (hh + 1) * DK]
            caug[32 * hh, DK] = float(S)
        in_maps.append(
            {
                "qT": qT,
                "kT": kT,
                "vT": vT,
                "maskT": maskT,
                "wp1": np.ascontiguousarray(wp1),
                "wp2": np.ascontiguousarray(wp2),
                "bq": np.ascontiguousarray(np.asarray(bq, np.float32)[ns, None]),
                "bk": np.ascontiguousarray(np.asarray(bk, np.float32)[ns, None]),
                "caug": caug.astype(BF16),
            }
        )
    return in_maps


def combine_outputs(results, Wv_bias, Wo, bo):
    """Sum per-core partial yT's (bf16 -> fp32), add host-folded biases."""
    acc = np.zeros((H, S), np.float32)
    for r in results:
        acc += r["yT"].astype(np.float32)
    bias = np.asarray(bo, np.float32) + np.asarray(Wv_bias, np.float32) @ np.asarray(
        Wo, np.float32
    ).T
    return (acc.T + bias[None, :]).astype(np.float32)[None]


def kernel(
    query,
    key,
    value,
    attention_mask,
    Wq,
    bq,
    Wk,
    bk,
    Wv,
    bv,
    Wo,
    bo,
    head,
    hidden_size,
):
    from concourse.bass_utils import run_bass_kernel_spmd

    nc = get_program()
    in_maps = make_in_maps(
        query, key, value, attention_mask, Wq, bq, Wk, bk, Wv, Wo
    )
    res = run_bass_kernel_spmd(nc, in_maps, list(range(NCORES)))
    return combine_outputs(res.results, bv, Wo, bo)


# revision 37
# speedup vs baseline: 1.0068x; 1.0068x over previous
"""Multi-head attention (B=1, S=2048, H=1024, NH=16) on 8 trn2 NeuronCores.

Sharding: head-parallel. Core c owns heads {2c, 2c+1} (= 128 of the 1024
hidden dims). Each core computes its Q/K/V projection slices, the full
attention for its 2 heads, and a full-width partial of the output
projection (contraction over its 128 context dims). Host sums the 8
partials and adds the (host-folded) biases.

v4 schedule - the Act engine is the pacemaker and nothing gates it:
  - The reference's masked softmax is exp(s*M) with masked scores set to
    0 (so masked weights are exp(0)=1).  We decompose
        exp(s*M) = (exp(s) - 1)*M + 1:
    Act computes plain exp(s/8) STRAIGHT FROM PSUM (no mask needed ->
    64 back-to-back 1024-wide exps, the critical 66us), DVE/Pool apply
    (E*M - M) in all-SBUF bf16 (DVE tensor_tensor runs 2x there), and
    the "+1" term enters the PV accumulation as a host-precomputed
    rank-1 matmul C = [colsum(V_proj) | 2048] per head.
  - q/k arrive as 512-token panels, projections panel-wise into one
    rotating PSUM bank (bias folded into evictions: Act for the first
    three panels, DVE after), k panels before masks (Act's S inputs).
  - GPSIMD cannot touch PSUM, so Pool only gets SBUF work: the tail of
    the mask-subtract, vaug ones columns.
  - per (h,j): S (4x512 PE into 2 psum tiles) -> 2 exps (Act) ->
    et=E*M (DVE 2x) -> et-=M (DVE [0:1152] + Pool [1152:2048]) ->
    PV (65-wide, ones-column denominator), software-pipelined by one j.
  - h0 epilogue (normalize) rides h1's slots; transposes/oT/y pipeline
    in the tail across aux/o-pool psum with Act+DVE evictions and eager
    per-chunk output DMA.

Precision: matmuls bf16 with fp32 PSUM accumulation; 0/1 mask bf16
(exact). Softmax runs without max-subtraction: exponent is (q.k/8) ~
N(0, 0.33^2) so exp never overflows; et = (E-1)*M is in [-1, 7].
"""

import math

import numpy as np
import ml_dtypes

BF16 = ml_dtypes.bfloat16
S, H, NH, DK = 2048, 1024, 16, 64
NCORES = 8
HPC = NH // NCORES          # heads per core = 2
DPC = HPC * DK              # head dims per core = 128
KC = H // 128               # contraction chunks = 8
TP = S // 512               # 512-wide token panels = 4
JC = S // 128               # 128-wide key chunks = 16
VA = DK + 1                 # v columns + ones column = 65
NWARM = 12                  # PE warm-up matmuls
DSUB = 1216                 # DVE's share of the mask-subtract columns

_CACHE = {}


def _oslc(ic):
    """o_ps column offset for ic-th 65-wide slice: 7 slices per 512-fp32
    PSUM bank so no matmul crosses a bank boundary."""
    b, r = divmod(ic, 7)
    return b * 512 + r * VA


def _build_program():
    """Build + compile the (identical) per-core Bass program."""
    from contextlib import ExitStack

    import concourse.bacc as bacc
    import concourse.bass as bass_mod
    import concourse.tile as tile
    from concourse import mybir

    dt = mybir.dt
    AF = mybir.ActivationFunctionType
    ALU = mybir.AluOpType

    nc = bacc.Bacc("TRN2", target_bir_lowering=False, debug=False)

    qT_d = nc.dram_tensor("qT", [H, S], dt.bfloat16, kind="ExternalInput").ap()
    kT_d = nc.dram_tensor("kT", [H, S], dt.bfloat16, kind="ExternalInput").ap()
    vT_d = nc.dram_tensor("vT", [H, S], dt.bfloat16, kind="ExternalInput").ap()
    maskT_d = nc.dram_tensor("maskT", [S, S], dt.bfloat16, kind="ExternalInput").ap()
    # wpack1 = [ident | wq | wk], wpack2 = [wv | wo]
    wp1_d = nc.dram_tensor("wp1", [128, 128 + 2 * KC * DPC], dt.bfloat16,
                           kind="ExternalInput").ap()
    wp2_d = nc.dram_tensor("wp2", [128, KC * DPC + H], dt.bfloat16,
                           kind="ExternalInput").ap()
    bq_d = nc.dram_tensor("bq", [DPC, 1], dt.float32, kind="ExternalInput").ap()
    bk_d = nc.dram_tensor("bk", [DPC, 1], dt.float32, kind="ExternalInput").ap()
    caug_d = nc.dram_tensor("caug", [33, VA], dt.bfloat16, kind="ExternalInput").ap()
    yT_d = nc.dram_tensor("yT", [H, S], dt.bfloat16, kind="ExternalOutput").ap()

    with tile.TileContext(nc) as tc, ExitStack() as ctx:
        cp = ctx.enter_context(tc.tile_pool(name="const", bufs=1))
        xin_p = ctx.enter_context(tc.tile_pool(name="xin", bufs=4))
        vin_p = ctx.enter_context(tc.tile_pool(name="vin", bufs=3))
        e_p = ctx.enter_context(tc.tile_pool(name="ex", bufs=12))
        ot_p = ctx.enter_context(tc.tile_pool(name="otok", bufs=2))
        rc_p = ctx.enter_context(tc.tile_pool(name="recip", bufs=3))
        y_p = ctx.enter_context(tc.tile_pool(name="ysb", bufs=6))
        # PSUM: aux 1 bank (warmup/proj/pv/tp/y-singles), s 2x2 banks
        # (S tiles, then y pairs), o 3 banks (PV acc, then tp scratch)
        aux_p = ctx.enter_context(tc.tile_pool(name="ps_aux", bufs=1, space="PSUM"))
        s_p = ctx.enter_context(tc.tile_pool(name="ps_s", bufs=2, space="PSUM"))
        o_p = ctx.enter_context(tc.tile_pool(name="ps_o", bufs=1, space="PSUM"))

        wp1 = cp.tile([128, 128 + 2 * KC * DPC], dt.bfloat16, tag="wp1")
        nc.sync.dma_start(out=wp1, in_=wp1_d)
        ident = wp1[:, 0:128]
        w_q = wp1[:, 128 : 128 + KC * DPC]
        w_k = wp1[:, 128 + KC * DPC : 128 + 2 * KC * DPC]
        bq_sb = cp.tile([DPC, 1], dt.float32, tag="bq")
        nc.sync.dma_start(out=bq_sb, in_=bq_d)
        bk_sb = cp.tile([DPC, 1], dt.float32, tag="bk")
        nc.sync.dma_start(out=bk_sb, in_=bk_d)
        caug_sb = cp.tile([33, VA], dt.bfloat16, tag="caug")
        nc.sync.dma_start(out=caug_sb, in_=caug_d)

        qT_sb = cp.tile([128, S], dt.bfloat16, tag="qTs")
        kT_sb = cp.tile([128, S], dt.bfloat16, tag="kTs")
        vaug = cp.tile([128, JC * (HPC * VA)], dt.bfloat16, tag="vaug")
        m_sb = [cp.tile([128, S], dt.bfloat16, tag=f"mj{j}", name=f"mj{j}")
                for j in range(JC)]
        oT_sb = [cp.tile([128, 512], dt.bfloat16, tag=f"oTp{p}", name=f"oTp{p}")
                 for p in range(TP)]
        ones1 = cp.tile([33, 128], dt.bfloat16, tag="ones1")
        nc.vector.memset(ones1, 1.0)

        xin = {}
        vin8 = [None] * KC

        def dma_panel(pre, x_d, p):
            xt = xin_p.tile([128, KC * 512], dt.bfloat16, tag="xin",
                            name=f"x{pre}{p}")
            nc.sync.dma_start(
                out=xt.rearrange("p (c i) -> p c i", c=KC),
                in_=x_d[:, p * 512 : (p + 1) * 512].rearrange(
                    "(c p) i -> p c i", p=128
                ),
            )
            xin[pre, p] = xt

        def dma_mask(j):
            nc.sync.dma_start(out=m_sb[j], in_=maskT_d[j * 128 : (j + 1) * 128, :])

        def dma_v(g):
            vt = vin_p.tile([128, KC * 256], dt.bfloat16, tag="vin", name=f"v{g}")
            nc.sync.dma_start(
                out=vt.rearrange("p (c i) -> p c i", c=KC),
                in_=vT_d[:, g * 256 : (g + 1) * 256].rearrange(
                    "(c p) i -> p c i", p=128
                ),
            )
            vin8[g] = vt

        # DMA order: all q/k panels first (they gate Act's exp stream),
        # then wv/wo, then masks and v tiles streaming (consumed by the
        # trailing DVE/Pool mask-apply + PV, which have slack).
        dma_panel("q", qT_d, 0)
        dma_panel("q", qT_d, 1)
        dma_panel("k", kT_d, 0)
        dma_panel("q", qT_d, 2)
        dma_panel("q", qT_d, 3)
        dma_panel("k", kT_d, 1)
        dma_panel("k", kT_d, 2)
        dma_panel("k", kT_d, 3)
        wp2 = cp.tile([128, KC * DPC + H], dt.bfloat16, tag="wp2")
        nc.sync.dma_start(out=wp2, in_=wp2_d)
        w_v = wp2[:, 0 : KC * DPC]
        wo_sb = wp2[:, KC * DPC : KC * DPC + H]
        dma_v(0)
        dma_mask(0)
        dma_mask(1)
        dma_v(1)
        dma_mask(2)
        dma_mask(3)
        dma_v(2)
        dma_mask(4)
        dma_mask(5)
        dma_v(3)
        dma_mask(6)
        dma_mask(7)
        dma_v(4)
        dma_mask(8)
        dma_mask(9)
        dma_v(5)
        dma_mask(10)
        dma_mask(11)
        dma_v(6)
        dma_mask(12)
        dma_mask(13)
        dma_v(7)
        dma_mask(14)
        dma_mask(15)

        # ones columns of vaug (Pool, SBUF-only)
        ones_cols = bass_mod.AP(
            tensor=vaug.tensor,
            offset=vaug.offset + DK,
            ap=[vaug.ap[0], [VA, JC * HPC], [1, 1]],
        )
        nc.gpsimd.memset(ones_cols, 1.0)

        # PE warm-up out of the cold p-state while DMAs stream
        for wi in range(NWARM):
            wps = aux_p.tile([128, 128], dt.float32, tag="aux", name=f"warm{wi}")
            nc.tensor.matmul(wps, lhsT=ident, rhs=ident, start=True, stop=True)

        # ---- panel-wise projections; first three evict on Act (they
        # precede the exp stream), later ones on DVE ----
        def proj_panel(pre, w_ap, b_sb, dest, p, act_evict):
            ps = aux_p.tile([128, 512], dt.float32, tag="aux", name=f"pp{pre}{p}")
            for kk in range(KC):
                nc.tensor.matmul(
                    ps,
                    lhsT=w_ap[:, kk * DPC : (kk + 1) * DPC],
                    rhs=xin[pre, p][:, kk * 512 : (kk + 1) * 512],
                    start=(kk == 0),
                    stop=(kk == KC - 1),
                )
            dst = dest[:, p * 512 : (p + 1) * 512]
            if act_evict:
                nc.scalar.activation(dst, ps, AF.Identity, bias=b_sb, scale=1.0)
            else:
                nc.vector.tensor_scalar(dst, ps, b_sb, None, ALU.add)

        def v_proj_chunk(t):
            ps = aux_p.tile([128, DPC], dt.float32, tag="aux", name=f"pv{t}")
            g, half = divmod(t, 2)
            for kk in range(KC):
                nc.tensor.matmul(
                    ps,
                    lhsT=vin8[g][:, kk * 256 + half * 128 : kk * 256 + half * 128 + 128],
                    rhs=w_v[:, kk * DPC : (kk + 1) * DPC],
                    start=(kk == 0),
                    stop=(kk == KC - 1),
                )
            # both heads' 64-col slices in one strided DVE copy
            base = t * (HPC * VA)
            dst = bass_mod.AP(
                tensor=vaug.tensor,
                offset=vaug.offset + base,
                ap=[vaug.ap[0], [VA, HPC], [1, DK]],
            )
            nc.vector.tensor_copy(dst, ps.rearrange("p (a d) -> p a d", d=DK))

        proj_panel("q", w_q, bq_sb, qT_sb, 0, True)
        proj_panel("q", w_q, bq_sb, qT_sb, 1, True)
        proj_panel("k", w_k, bk_sb, kT_sb, 0, True)

        # ---- attention ----
        import itertools

        def emit_exps(h, j, et):
            """S^T chunk j (PE) + the two 1024-wide exps (Act, from PSUM)."""
            hs = h * DK
            for half in range(2):
                ps = s_p.tile([128, 1024], dt.float32, tag="sps",
                              name=f"s{h}_{j}_{half}")
                for q in range(2):
                    pi = half * 2 + q
                    nc.tensor.matmul(
                        ps[:, q * 512 : (q + 1) * 512],
                        lhsT=kT_sb[hs : hs + DK, j * 128 : (j + 1) * 128],
                        rhs=qT_sb[hs : hs + DK, pi * 512 : (pi + 1) * 512],
                        start=True,
                        stop=True,
                    )
                nc.scalar.activation(
                    et[:, half * 1024 : (half + 1) * 1024], ps, AF.Exp,
                    scale=1.0 / math.sqrt(DK),
                )

        def emit_mask(h, j, et):
            """et = et*M - M, all-SBUF bf16 (DVE tensor_tensor runs 2x).
            Pool takes more of the subtract during h1 so DVE never trails
            the exp stream at the end."""
            mj = m_sb[j]
            ds = 1200
            nc.vector.tensor_tensor(et, et, mj, ALU.mult)
            nc.vector.tensor_tensor(
                et[:, 0:ds], et[:, 0:ds], mj[:, 0:ds], ALU.subtract
            )
            nc.gpsimd.tensor_tensor(
                et[:, ds:S], et[:, ds:S], mj[:, ds:S], ALU.subtract
            )

        def pv_c(h, o_ps):
            """+1 correction: rank-1 C rows into every o slice."""
            for ic in range(JC):
                nc.tensor.matmul(
                    o_ps[:, _oslc(ic) : _oslc(ic) + VA],
                    lhsT=ones1[32 * h : 32 * h + 1, :],
                    rhs=caug_sb[32 * h : 32 * h + 1, :],
                    start=(ic % 7 == 0),
                    stop=False,
                )

        def pv_mms(h, j, et, o_ps):
            for ic in range(JC):
                nc.tensor.matmul(
                    o_ps[:, _oslc(ic) : _oslc(ic) + VA],
                    lhsT=et[:, ic * 128 : (ic + 1) * 128],
                    rhs=vaug[:, j * (HPC * VA) + h * VA : j * (HPC * VA) + (h + 1) * VA],
                    start=False,
                    stop=(j == JC - 1 and (ic % 7 == 6 or ic == JC - 1)),
                )

        epi_q = []

        def norm_bank(h, o_ps, b, ot_big):
            n_ic = (7, 7, 2)[b]
            rc = rc_p.tile([128, 8], dt.float32, tag="rc", name=f"rc{h}_{b}")
            den = bass_mod.AP(
                tensor=o_ps.tensor,
                offset=o_ps.offset + b * 512 + DK,
                ap=[o_ps.ap[0], [VA, n_ic]],
            )
            nc.vector.reciprocal(rc[:, :n_ic], den)
            src_ap = bass_mod.AP(
                tensor=o_ps.tensor,
                offset=o_ps.offset + b * 512,
                ap=[o_ps.ap[0], [VA, n_ic], [1, DK]],
            )
            rcb = bass_mod.AP(
                tensor=rc.tensor,
                offset=rc.offset,
                ap=[rc.ap[0], [1, n_ic], [0, DK]],
            )
            dst = ot_big[:, b * 7 * DK : (b * 7 + n_ic) * DK].rearrange(
                "p (a d) -> p a d", d=DK
            )
            nc.vector.tensor_mul(dst, src_ap, rcb)

        def tp_pair(h, ic, ot_big, tp_dst, dve_both=False):
            """Transpose O chunk pair (ic, ic+1) [128,128]; copy the two
            64-row halves into oT_sb (DVE + Act, or both DVE when riding
            h1's slots where Act is the pacemaker)."""
            hs = h * DK
            ot = ot_big[:, ic * DK : (ic + 2) * DK]
            nc.tensor.transpose(tp_dst, ot, ident)
            p = ic // 4
            dst0 = oT_sb[p][hs : hs + DK, (ic % 4) * 128 : (ic % 4 + 1) * 128]
            dst1 = oT_sb[p][hs : hs + DK, (ic % 4 + 1) * 128 : (ic % 4 + 2) * 128]
            nc.vector.tensor_copy(dst0, tp_dst[0:DK, :])
            if dve_both:
                nc.vector.tensor_copy(dst1, tp_dst[DK : 2 * DK, :])
            else:
                nc.scalar.activation(dst1, tp_dst[DK : 2 * DK, :], AF.Copy)

        def tp_pair_aux0(ic):
            tp = aux_p.tile([128, 128], dt.bfloat16, tag="aux", name=f"tpe0_{ic}")
            tp_pair(0, ic, ot_bigs[0], tp, dve_both=True)

        def emit_exp_half(h, j, et, half):
            hs = h * DK
            ps = s_p.tile([128, 1024], dt.float32, tag="sps",
                          name=f"s{h}_{j}_{half}")
            for q in range(2):
                pi = half * 2 + q
                nc.tensor.matmul(
                    ps[:, q * 512 : (q + 1) * 512],
                    lhsT=kT_sb[hs : hs + DK, j * 128 : (j + 1) * 128],
                    rhs=qT_sb[hs : hs + DK, pi * 512 : (pi + 1) * 512],
                    start=True,
                    stop=True,
                )
            nc.scalar.activation(
                et[:, half * 1024 : (half + 1) * 1024], ps, AF.Exp,
                scale=1.0 / math.sqrt(DK),
            )

        o_ps_h = [None, None]
        ot_bigs = [None, None]
        for h in range(HPC):
            o_ps = o_p.tile([128, 1536], dt.float32, tag="ops", name=f"ops{h}")
            o_ps_h[h] = o_ps
            pv_c(h, o_ps)
            pend = []  # (j, et) with exps emitted, mask+PV pending
            if h == 0:
                # warm start: half0 of j0/j1 (need only q panels 0-1 + k0),
                # then the deferred projections ride between the exps
                ets = {}
                for j in (0, 1):
                    ets[j] = e_p.tile([128, S], dt.bfloat16, tag="et",
                                      name=f"et0_{j}")
                    emit_exp_half(0, j, ets[j], 0)
                proj_panel("q", w_q, bq_sb, qT_sb, 2, True)
                proj_panel("q", w_q, bq_sb, qT_sb, 3, True)
                for j in (0, 1):
                    emit_exp_half(0, j, ets[j], 1)
                    pend.append((j, ets[j]))
                proj_panel("k", w_k, bk_sb, kT_sb, 1, False)
                proj_panel("k", w_k, bk_sb, kT_sb, 2, False)
                proj_panel("k", w_k, bk_sb, kT_sb, 3, False)
                jstart = 2
            else:
                jstart = 0
            for j in range(jstart, JC):
                et = e_p.tile([128, S], dt.bfloat16, tag="et", name=f"et{h}_{j}")
                emit_exps(h, j, et)
                if pend:
                    pj, pet = pend.pop(0)
                    emit_mask(h, pj, pet)
                    if h == 0:
                        v_proj_chunk(pj)
                    else:
                        if epi_q:
                            epi_q.pop(0)()
                    pv_mms(h, pj, pet, o_ps)
                pend.append((j, et))
            for pj, pet in pend:
                emit_mask(h, pj, pet)
                if h == 0:
                    v_proj_chunk(pj)
                pv_mms(h, pj, pet, o_ps)
            ot_bigs[h] = ot_p.tile([128, JC * DK], dt.bfloat16, tag="ot",
                                   name=f"otb{h}")
            if h == 0:
                for b in range(3):
                    epi_q.append(
                        lambda b=b: norm_bank(0, o_ps_h[0], b, ot_bigs[0])
                    )
                for ic in range(0, JC, 2):
                    epi_q.append(lambda ic=ic: tp_pair_aux0(ic))
            else:
                while epi_q:
                    epi_q.pop(0)()
                for b in range(3):
                    norm_bank(1, o_ps_h[1], b, ot_bigs[1])

        # ---- tail: per panel-half, transposes for both heads (aux /
        # o-scratch alternating psum, copies split DVE/Act) immediately
        # followed by that half's y chunks over 4 psum streams ----
        flip = itertools.cycle((True, False))
        tpi = 0

        def do_tp(h, ic):
            if (ic // 2 + h) % 2 == 0:
                tp = aux_p.tile([128, 128], dt.bfloat16, tag="aux",
                                name=f"tp{h}_{ic}")
            else:
                tp = o_p.tile([128, 128], dt.bfloat16, tag="ops",
                              name=f"tp{h}_{ic}")
            tp_pair(h, ic, ot_bigs[h], tp)

        for ph in range(2):
            for ic in range(8 * ph, 8 * ph + 8, 2):
                do_tp(1, ic)
            for nn in range(KC):
                ysb = y_p.tile([128, 1024], dt.bfloat16, tag="ysb",
                               name=f"ysb{ph}_{nn}")
                if nn % 4 == 3:
                    # singles on aux + o-pool rotations
                    for pp in range(2):
                        if pp == 0:
                            y_ps = aux_p.tile([128, 512], dt.float32,
                                              tag="aux", name=f"ya{ph}_{nn}")
                        else:
                            y_ps = o_p.tile([128, 512], dt.float32,
                                            tag="ops", name=f"yo{ph}_{nn}")
                        nc.tensor.matmul(
                            y_ps,
                            lhsT=wo_sb[:, nn * 128 : (nn + 1) * 128],
                            rhs=oT_sb[ph * 2 + pp],
                            start=True,
                            stop=True,
                        )
                        if next(flip):
                            nc.scalar.activation(
                                ysb[:, pp * 512 : (pp + 1) * 512], y_ps, AF.Copy
                            )
                        else:
                            nc.vector.tensor_copy(
                                ysb[:, pp * 512 : (pp + 1) * 512], y_ps
                            )
                else:
                    y_ps = s_p.tile([128, 1024], dt.float32, tag="sps",
                                    name=f"y{ph}_{nn}")
                    for pp in range(2):
                        nc.tensor.matmul(
                            y_ps[:, pp * 512 : (pp + 1) * 512],
                            lhsT=wo_sb[:, nn * 128 : (nn + 1) * 128],
                            rhs=oT_sb[ph * 2 + pp],
                            start=True,
                            stop=True,
                        )
                    if next(flip):
                        nc.scalar.activation(ysb, y_ps, AF.Copy)
                    else:
                        nc.vector.tensor_copy(ysb, y_ps)
                nc.sync.dma_start(
                    out=yT_d[nn * 128 : (nn + 1) * 128,
                             ph * 1024 : ph * 1024 + 1024],
                    in_=ysb,
                )

    nc.compile()
    return nc


def get_program():
    if "nc" not in _CACHE:
        _CACHE["nc"] = _build_program()
    return _CACHE["nc"]


def _wshuf(wT):
    """[1024 k, 128 n] -> [128 p, KC*128] with chunk kk at cols kk*128."""
    return np.ascontiguousarray(
        wT.reshape(KC, 128, DPC).transpose(1, 0, 2).reshape(128, KC * DPC)
    ).astype(BF16)


def make_in_maps(query, key, value, attention_mask, Wq, bq, Wk, bk, Wv, Wo):
    """Host-side sharding: per-core input dicts."""
    qT = np.ascontiguousarray(np.asarray(query, np.float32)[0].T).astype(BF16)
    kT = np.ascontiguousarray(np.asarray(key, np.float32)[0].T).astype(BF16)
    vT = np.ascontiguousarray(np.asarray(value, np.float32)[0].T).astype(BF16)
    maskT = np.ascontiguousarray(
        np.asarray(attention_mask, np.float32)[0, 0].T
    ).astype(BF16)
    v32 = np.asarray(value, np.float32)[0]
    vcol = v32.sum(axis=0)  # [H]

    in_maps = []
    for c in range(NCORES):
        ns = slice(c * DPC, (c + 1) * DPC)
        wq = _wshuf(np.asarray(Wq, np.float32)[ns].T)
        wk = _wshuf(np.asarray(Wk, np.float32)[ns].T)
        wv = _wshuf(np.asarray(Wv, np.float32)[ns].T)
        wo = np.ascontiguousarray(np.asarray(Wo, np.float32)[:, ns].T).astype(BF16)
        wp1 = np.concatenate([np.eye(128, dtype=BF16), wq, wk], axis=1)
        wp2 = np.concatenate([wv, wo], axis=1)
        cvec = vcol @ np.asarray(Wv, np.float32)[ns].T  # [DPC]
        caug = np.zeros((33, VA), np.float32)
        for hh in range(HPC):
            caug[32 * hh, 0:DK] = cvec[hh * DK : (hh + 1) * DK]
            caug[32 * hh, DK] = float(S)
        in_maps.append(
            {
                "qT": qT,
                "kT": kT,
                "vT": vT,
                "maskT": maskT,
                "wp1": np.ascontiguousarray(wp1),
                "wp2": np.ascontiguousarray(wp2),
                "bq": np.ascontiguousarray(np.asarray(bq, np.float32)[ns, None]),
                "bk": np.ascontiguousarray(np.asarray(bk, np.float32)[ns, None]),
                "caug": caug.astype(BF16),
            }
        )
    return in_maps


def combine_outputs(results, Wv_bias, Wo, bo):
    """Sum per-core partial yT's (bf16 -> fp32), add host-folded biases."""
    acc = np.zeros((H, S), np.float32)
    for r in results:
        acc += r["yT"].astype(np.float32)
    bias = np.asarray(bo, np.float32) + np.asarray(Wv_bias, np.float32) @ np.asarray(
        Wo, np.float32
    ).T
    return (acc.T + bias[None, :]).astype(np.float32)[None]


def kernel(
    query,
    key,
    value,
    attention_mask,
    Wq,
    bq,
    Wk,
    bk,
    Wv,
    bv,
    Wo,
    bo,
    head,
    hidden_size,
):
    from concourse.bass_utils import run_bass_kernel_spmd

    nc = get_program()
    in_maps = make_in_maps(
        query, key, value, attention_mask, Wq, bq, Wk, bk, Wv, Wo
    )
    res = run_bass_kernel_spmd(nc, in_maps, list(range(NCORES)))
    return combine_outputs(res.results, bv, Wo, bo)
